# revision 19
# baseline (speedup 1.0000x reference)
"""CRF Viterbi decode (torchcrf semantics) on 8 Trainium2 NeuronCores.

Strategy: pure data parallel over batch (1024 rows -> 128 rows/core, one row
per SBUF partition).  Per core:

  Forward (DVE only, scores resident in SBUF, bit-exact vs the jax ref):
    cand[b,j,i] = score[b,i] + trans[i,j]   (stride-0 broadcast TT add)
    premax[b,j] = max_i cand[b,j,i]         (segmented tensor_reduce)
    score'[b,j] = premax[b,j] + em[b,t,j]   (small TT add)
  This 3-op chain is the DVE floor: neuronxcc rejects generic compute on the
  Pool engine, the Activation engine only takes [P,1] bias vectors, fp32 PE
  matmuls run at 4 cycles/row, and no DVE op fuses a tensor-tensor stage
  with a segmented reduce.  Any further op-splitting loses to the ~95ns
  per-op overhead plus the full-vector barrier each step carries.

  Backward (recomputes each step's candidates instead of storing bp):
    A 64-wide block-diagonal one-hot of tag_{s+1} (iota64 holds 0..31 at
    free offset 32*(r%2) for row-block r, 255 elsewhere) is block-transposed
    by the 32x32 vector-transpose, so TWO 64-contraction tile_position
    matmuls gather transsel[b,i] = trans[i, tag_{s+1}(b)] (vs four 32-wide).
    The max value needed by max_index is not recomputed: it equals
    hist_{s+1}[b, tag_{s+1}] bitwise (max-then-add-const == add-const-then-
    max for fp max VALUES), so the per-row scalars are gathered by two DVE
    TTR ops that run inside the PE window:
      emneg[b]  = sum (onehot * em_{s+1-replicated}) * -1
      histsel[b] = sum (onehot * hist_{s+1-replicated}) -> m8 slot 0
    tmp = (hist_s - transsel*(-1) - emneg)*1  (ln_bwd_dx; associations match
    the ref exactly), then max_index against histsel (first-index tie break
    = jnp.argmax).  All max_index inputs are same-engine, so it dispatches
    in-order with no cross-engine event-semaphore wait.

Inputs are taken at full shape; sharding/gather happens on host inside
kernel().
"""

import sys

import numpy as np

if "/opt/trn_rl_repo" not in sys.path:
    sys.path.insert(0, "/opt/trn_rl_repo")

B, T, K = 1024, 1024, 32
NCORES = 8
BL = B // NCORES  # 128 batch rows per core
TC = 128  # time chunk (em streaming / tags8 chunking)
POS_BIG = 3.0e38

# forward j-split: DVE owns j in [0, JD), Pool owns j in [JD, K).
# NOTE: neuronxcc rejects generic compute (TensorTensor/TensorScalarPtr) on
# the Pool engine, so jpool must stay 0 on real hardware; the split code is
# kept for cost-model experiments only.
JPOOL = 0


def build_nc(t_steps: int = T, tc: int = TC, jpool: int = JPOOL):
    """Build + compile the per-core Bass program (same NEFF on all 8 cores)."""
    import concourse.bass as bass
    import concourse.tile as tile
    from concourse import bacc, mybir

    f32 = mybir.dt.float32
    u32 = mybir.dt.uint32
    i32 = mybir.dt.int32
    Alu = mybir.AluOpType
    Ax = mybir.AxisListType

    nsteps = t_steps
    nchunks = (nsteps + tc - 1) // tc
    assert nsteps % tc == 0

    nc = bacc.Bacc(
        "TRN2", target_bir_lowering=False, debug=False, enable_asserts=False
    )

    em_d = nc.dram_tensor("em", [BL, nsteps * K], f32, kind="ExternalInput").ap()
    ttb_d = nc.dram_tensor("ttb", [BL, K * K], f32, kind="ExternalInput").ap()
    tmov_d = nc.dram_tensor("tmov", [128, K], f32, kind="ExternalInput").ap()
    endt_d = nc.dram_tensor("endt", [BL, K], f32, kind="ExternalInput").ap()
    iota_d = nc.dram_tensor("iota", [BL, K], u32, kind="ExternalInput").ap()
    # block-diagonal iota: row-block r holds 0..31 at free offset 32*(r%2),
    # 255 elsewhere -> onehot64 is block-diagonal, so one 64-contraction
    # matmul covers two row blocks (2 matmuls/step instead of 4)
    iota64_d = nc.dram_tensor("iota64", [BL, 2 * K], u32, kind="ExternalInput").ap()
    tags_d = nc.dram_tensor("tags", [BL, nsteps], i32, kind="ExternalOutput").ap()

    with tile.TileContext(nc) as tc_ctx:
        _body(nc, tc_ctx, bass, mybir, Alu, Ax, f32, u32, i32,
              em_d, ttb_d, tmov_d, endt_d, iota_d, iota64_d, tags_d, nsteps,
              tc, nchunks, jpool)

    nc.compile()
    return nc


def _body(nc, tc_ctx, bass, mybir, Alu, Ax, f32, u32, i32,
          em_d, ttb_d, tmov_d, endt_d, iota_d, iota64_d, tags_d, nsteps,
          tc, nchunks, jpool):
    from contextlib import ExitStack

    from concourse.dve_ops import TENSOR_TENSOR_REDUCE as _CTTR

    jd = K - jpool  # DVE-owned j count

    ctx = ExitStack()
    with ctx:
        const_pool = ctx.enter_context(tc_ctx.tile_pool(name="const", bufs=1))
        hist_pool = ctx.enter_context(tc_ctx.tile_pool(name="hist", bufs=1))
        em_pool = ctx.enter_context(tc_ctx.tile_pool(name="em", bufs=2))
        work_pool = ctx.enter_context(tc_ctx.tile_pool(name="work", bufs=1))
        tags8_pool = ctx.enter_context(tc_ctx.tile_pool(name="tags8", bufs=2))
        psum_pool = ctx.enter_context(
            tc_ctx.tile_pool(name="psum", bufs=2, space="PSUM")
        )

        # ---- constants ----
        ttb = const_pool.tile([BL, K * K], f32)  # ttb[b, j*K+i] = trans[i, j]
        nc.sync.dma_start(ttb[:], ttb_d[:])
        tmov = const_pool.tile([128, K], f32)  # trans.T tiled x4 over partitions
        nc.sync.dma_start(tmov[:], tmov_d[:])
        endt = const_pool.tile([BL, K], f32)
        nc.sync.dma_start(endt[:], endt_d[:])
        iota = const_pool.tile([BL, K], u32)
        nc.sync.dma_start(iota[:], iota_d[:])
        iota64 = const_pool.tile([BL, 2 * K], u32)
        nc.sync.dma_start(iota64[:], iota64_d[:])

        # ---- working tiles ----
        hist = hist_pool.tile([BL, nsteps * K], f32)  # all forward scores
        cand = work_pool.tile([BL, max(jd, 1) * K], f32)  # DVE j-slice
        candp = work_pool.tile([BL, max(jpool, 1) * K], f32)  # Pool j-slice
        l1 = work_pool.tile([BL, max(jpool, 1) * (K // 2)], f32)
        premax = work_pool.tile([BL, K], f32)
        # double-buffered by step parity: breaks the Pool-write-after-DVE-read
        # serialization on the per-step scalar tiles
        m8s = [work_pool.tile([BL, 8], f32, name=f"m8_{i}") for i in range(2)]
        tmps = [work_pool.tile([BL, K], f32, name=f"tmp_{i}") for i in range(2)]
        emnegs = [work_pool.tile([BL, 1], f32, name=f"emneg_{i}") for i in range(2)]
        scrs = [work_pool.tile([BL, 2 * K], f32, name=f"scr_{i}") for i in range(2)]
        scr2s = [work_pool.tile([BL, 2 * K], f32, name=f"scr2_{i}") for i in range(2)]
        onehots = [work_pool.tile([BL, 2 * K], f32, name=f"oh_{i}") for i in range(2)]
        vts = [work_pool.tile([BL, 2 * K], f32, name=f"vt_{i}") for i in range(2)]
        tagout = work_pool.tile([BL, nsteps], i32)

        nc.vector.memset(m8s[0][:], POS_BIG)
        nc.vector.memset(m8s[1][:], POS_BIG)

        ttb3 = ttb[:].rearrange("p (j i) -> p j i", i=K)
        cand3 = cand[:].rearrange("p (j i) -> p j i", i=K)
        candp3 = candp[:].rearrange("p (j i) -> p j i", i=K)
        l13 = l1[:].rearrange("p (j i) -> p j i", i=K // 2)

        # ================= forward =================
        for c in range(nchunks):
            emf = em_pool.tile([BL, tc * K], f32, tag="emchunk")
            nc.sync.dma_start(emf[:], em_d[:, c * tc * K : (c + 1) * tc * K])
            for tloc in range(tc):
                t = c * tc + tloc
                em_sl = emf[:, tloc * K : (tloc + 1) * K]
                h_t = hist[:, t * K : (t + 1) * K]
                if t == 0:
                    nc.vector.tensor_copy(h_t, em_sl)
                    continue
                h_prev = hist[:, (t - 1) * K : t * K]
                h_bc = h_prev[:, None, :]
                if jpool > 0:
                    # Pool: cand slice [jd, K) + pairwise L1 max to 16-wide
                    nc.gpsimd.scalar_tensor_tensor(
                        candp3[:, 0:jpool, :],
                        h_bc.broadcast_to([BL, jpool, K]),
                        0.0,
                        ttb3[:, jd:K, :],
                        Alu.bypass,
                        Alu.add,
                    )
                    nc.gpsimd.tensor_tensor(
                        l13[:, 0:jpool, :],
                        candp3[:, 0:jpool, 0 : K // 2],
                        candp3[:, 0:jpool, K // 2 : K],
                        Alu.max,
                    )
                if jd > 0:
                    nc.vector.tensor_tensor(
                        cand3[:, 0:jd, :],
                        h_bc.broadcast_to([BL, jd, K]),
                        ttb3[:, 0:jd, :],
                        Alu.add,
                    )
                    nc.vector.tensor_reduce(
                        premax[:, 0:jd], cand3[:, 0:jd, :], Ax.X, Alu.max
                    )
                if jpool > 0:
                    nc.vector.tensor_reduce(
                        premax[:, jd:K], l13[:, 0:jpool, :], Ax.X, Alu.max
                    )
                nc.vector.tensor_tensor(h_t, premax[:], em_sl, Alu.add)

        # ================= final argmax =================
        # ref: score = hist[T-1] + end_transitions, then argmax (first index)
        tags8_cur = tags8_pool.tile([BL, tc * 8], u32, tag="t8")
        tmp0 = tmps[(nsteps - 1) % 2]
        m80 = m8s[(nsteps - 1) % 2]
        nc.vector.tensor_tensor(
            tmp0[:], hist[:, (nsteps - 1) * K : nsteps * K], endt[:], Alu.add
        )
        nc.vector.tensor_reduce(m80[:, 0:1], tmp0[:], Ax.X, Alu.max)
        last_slot = (nsteps - 1) - (nchunks - 1) * tc
        nc.vector.max_index(
            tags8_cur[:, last_slot * 8 : last_slot * 8 + 8], m80[:], tmp0[:]
        )

        # ================= backward =================
        tags8_by_chunk = {nchunks - 1: tags8_cur}
        for c in range(nchunks - 1, -1, -1):
            # em[s+1] for s in [c*tc, (c+1)*tc): dram slice offset by one step
            n_em = tc if c < nchunks - 1 else tc - 1
            embw = em_pool.tile([BL, tc * K], f32, tag="emchunk")
            nc.sync.dma_start(
                embw[:, : n_em * K],
                em_d[:, (c * tc + 1) * K : (c * tc + 1 + n_em) * K],
            )
            if c not in tags8_by_chunk:
                tags8_by_chunk[c] = tags8_pool.tile(
                    [BL, tc * 8], u32, tag="t8", name=f"t8c{c}"
                )
            t8c = tags8_by_chunk[c]

            s_hi = min(nsteps - 2, (c + 1) * tc - 1)
            for s in range(s_hi, c * tc - 1, -1):
                tloc = s - c * tc
                par = s % 2
                onehot, vt = onehots[par], vts[par]
                tmp, m8, emneg = tmps[par], m8s[par], emnegs[par]
                # one-hot of tag_{s+1}
                sp1 = s + 1
                cp1 = sp1 // tc
                t8p = tags8_by_chunk[cp1]
                slot = sp1 - cp1 * tc
                # block-diagonal one-hot of tag_{s+1} ([BL, 64]; row-block r
                # has its 32 valid lanes at free offset 32*(r%2), zeros
                # elsewhere since iota64 is 255 there)
                nc.vector.tensor_tensor(
                    onehot[:],
                    iota64[:],
                    t8p[:, slot * 8 : slot * 8 + 1].broadcast_to([BL, 2 * K]),
                    Alu.is_equal,
                )
                nc.vector.transpose(vt[:], onehot[:])
                # per-row scalars (DVE TTRs, run inside the PE window), using
                # the j-replicated view of em/hist so the 255-half contributes
                # exact zeros:
                # emneg[b] = -em_{s+1}[b, tag];
                # histsel[b] = hist_{s+1}[b, tag] -> m8 slot 0 (bitwise equal
                # to max(tmp), so the separate tensor_reduce is not needed, and
                # max_index has no cross-engine dep so it dispatches in-order)
                oh3 = onehot[:].rearrange("p (c i) -> p c i", i=K)
                em_sl_bw = embw[:, tloc * K : (tloc + 1) * K]
                nc.vector._custom_dve(
                    _CTTR,
                    out=scrs[par][:].rearrange("p (c i) -> p c i", i=K),
                    in0=oh3,
                    in1=em_sl_bw[:, None, :].broadcast_to([BL, 2, K]),
                    s0=0.0,
                    s1=-1.0,
                    accum_out=emneg[:],
                )
                nc.vector._custom_dve(
                    _CTTR,
                    out=scr2s[par][:].rearrange("p (c i) -> p c i", i=K),
                    in0=oh3,
                    in1=hist[:, sp1 * K : (sp1 + 1) * K][:, None, :]
                    .broadcast_to([BL, 2, K]),
                    s0=0.0,
                    s1=1.0,
                    accum_out=m8[:, 0:1],
                )
                # transsel[b,i] = trans[i, tag_b] via 2 diagonal 64x64 matmuls
                tsel = psum_pool.tile([BL, K], f32, tag="tsel")
                for r in range(2):
                    nc.tensor.matmul(
                        tsel[64 * r : 64 * r + 64, :],
                        vt[64 * r : 64 * r + 64, :],
                        tmov[64 * r : 64 * r + 64, :],
                        start=True,
                        stop=True,
                        tile_position=(64 * r, 64 * r),
                    )
                # tmp = (hist_s - tsel*(-1) - emneg)*1 = (hist_s+tsel)+em
                # -- bitwise identical to the ref association (sign flips and
                # subtract-of-negation are IEEE-exact).  The max value for
                # max_index is hist_{s+1}[tag] (bitwise == max(tmp)).
                nc.vector.ln_bwd_dx(
                    tmp[:], hist[:, s * K : (s + 1) * K], tsel[:], -1.0,
                    emneg[:], 1.0,
                )
                nc.vector.max_index(
                    t8c[:, tloc * 8 : tloc * 8 + 8], m8[:], tmp[:]
                )

            # compact this chunk's tags (slot stride 8 -> dense) on ScalarE
            t83 = t8c[:].rearrange("p (s e) -> p s e", e=8)
            nc.scalar.copy(
                tagout[:, c * tc : (c + 1) * tc][:, :, None], t83[:, :, 0:1]
            )
            nc.sync.dma_start(
                tags_d[:, c * tc : (c + 1) * tc], tagout[:, c * tc : (c + 1) * tc]
            )
            if c + 1 in tags8_by_chunk:
                del tags8_by_chunk[c + 1]


_NC_CACHE = {}


def _get_nc(t_steps=T, tc=TC, jpool=JPOOL):
    key = (t_steps, tc, jpool)
    if key not in _NC_CACHE:
        _NC_CACHE[key] = build_nc(t_steps, tc, jpool)
    return _NC_CACHE[key]


def make_in_maps(inputs, start_transitions, end_transitions, transitions,
                 t_steps=T):
    """Host-side shard + constant prep. Returns list of per-core input dicts."""
    inputs = np.asarray(inputs, np.float32)
    start = np.asarray(start_transitions, np.float32)
    end = np.asarray(end_transitions, np.float32)
    trans = np.asarray(transitions, np.float32)

    ttb = np.ascontiguousarray(
        np.broadcast_to(trans.T.reshape(1, K * K), (BL, K * K))
    )
    tmov = np.ascontiguousarray(np.tile(trans.T, (4, 1)))
    endt = np.ascontiguousarray(np.broadcast_to(end.reshape(1, K), (BL, K)))
    iota = np.ascontiguousarray(
        np.broadcast_to(np.arange(K, dtype=np.uint32), (BL, K))
    )
    # block-diagonal iota for the 64-wide onehot: row-block r holds 0..31 at
    # free offset 32*(r%2), 255 (never a tag) elsewhere
    iota64 = np.full((BL, 2 * K), 255, dtype=np.uint32)
    for r in range(BL // K):
        off = K * (r % 2)
        iota64[r * K : (r + 1) * K, off : off + K] = np.arange(K, dtype=np.uint32)

    in_maps = []
    for ci in range(NCORES):
        em = np.array(
            inputs[ci * BL : (ci + 1) * BL, :t_steps].reshape(BL, t_steps * K)
        )
        # fold start_transitions into em[0] (same association as the ref)
        em[:, :K] = start.reshape(1, K) + em[:, :K]
        in_maps.append(
            {"em": em, "ttb": ttb, "tmov": tmov, "endt": endt, "iota": iota,
             "iota64": iota64}
        )
    return in_maps


_last_result = None


def kernel(inputs, mask, start_transitions, end_transitions, transitions):
    global _last_result
    mask = np.asarray(mask)
    if not mask.all():
        return _numpy_fallback(
            np.asarray(inputs, np.float32), mask,
            np.asarray(start_transitions, np.float32),
            np.asarray(end_transitions, np.float32),
            np.asarray(transitions, np.float32),
        )

    from concourse.bass_utils import run_bass_kernel_spmd

    nc = _get_nc()
    in_maps = make_in_maps(inputs, start_transitions, end_transitions, transitions)
    res = run_bass_kernel_spmd(nc, in_maps, core_ids=list(range(NCORES)))
    _last_result = res
    tags = np.concatenate([res.results[i]["tags"] for i in range(NCORES)], axis=0)
    return tags.astype(np.int32)


def _numpy_fallback(inputs, mask, start, end, trans):
    """Vectorized numpy Viterbi matching torchcrf/ref semantics (general mask)."""
    em = np.swapaxes(inputs, 0, 1)  # [T, B, K]
    mk = np.swapaxes(mask, 0, 1)  # [T, B]
    nT, nB, nK = em.shape
    score = start[None, :] + em[0]
    hist = np.zeros((nT - 1, nB, nK), np.int32)
    for t in range(1, nT):
        cand = score[:, :, None] + trans[None, :, :] + em[t][:, None, :]
        bp = np.argmax(cand, axis=1).astype(np.int32)
        ns = np.max(cand, axis=1)
        m = mk[t][:, None]
        score = np.where(m, ns, score)
        hist[t - 1] = bp
    score = score + end[None, :]
    tag = np.argmax(score, axis=1).astype(np.int32)
    tags = np.zeros((nT, nB), np.int32)
    tags[nT - 1] = tag
    for t in range(nT - 2, -1, -1):
        prev = np.take_along_axis(hist[t], tag[:, None], axis=1)[:, 0]
        prev = np.where(mk[t + 1], prev, tag)
        tags[t] = prev
        tag = prev
    return np.swapaxes(tags, 0, 1).astype(np.int32)


# revision 20
# speedup vs baseline: 1.0005x; 1.0005x over previous
"""CRF Viterbi decode (torchcrf semantics) on 8 Trainium2 NeuronCores.

Strategy: pure data parallel over batch (1024 rows -> 128 rows/core, one row
per SBUF partition).  Per core:

  Forward (DVE only, scores resident in SBUF, bit-exact vs the jax ref):
    cand[b,j,i] = score[b,i] + trans[i,j]   (stride-0 broadcast TT add)
    premax[b,j] = max_i cand[b,j,i]         (segmented tensor_reduce)
    score'[b,j] = premax[b,j] + em[b,t,j]   (small TT add)
  This 3-op chain is the DVE floor: neuronxcc rejects generic compute on the
  Pool engine, the Activation engine only takes [P,1] bias vectors, fp32 PE
  matmuls run at 4 cycles/row, and no DVE op fuses a tensor-tensor stage
  with a segmented reduce.  Any further op-splitting loses to the ~95ns
  per-op overhead plus the full-vector barrier each step carries.

  Backward (recomputes each step's candidates instead of storing bp):
    A 64-wide block-diagonal one-hot of tag_{s+1} (iota64 holds 0..31 at
    free offset 32*(r%2) for row-block r, 255 elsewhere) is block-transposed
    by the 32x32 vector-transpose, so TWO 64-contraction tile_position
    matmuls gather transsel[b,i] = trans[i, tag_{s+1}(b)] (vs four 32-wide).
    The max value needed by max_index is not recomputed: it equals
    hist_{s+1}[b, tag_{s+1}] bitwise (max-then-add-const == add-const-then-
    max for fp max VALUES), so the per-row scalars are gathered by two DVE
    TTR ops that run inside the PE window:
      emneg[b]  = sum (onehot * em_{s+1-replicated}) * -1
      histsel[b] = sum (onehot * hist_{s+1-replicated}) -> m8 slot 0
    tmp = (hist_s - transsel*(-1) - emneg)*1  (ln_bwd_dx; associations match
    the ref exactly), then max_index against histsel (first-index tie break
    = jnp.argmax).  All max_index inputs are same-engine, so it dispatches
    in-order with no cross-engine event-semaphore wait.

Inputs are taken at full shape; sharding/gather happens on host inside
kernel().
"""

import sys

import numpy as np

if "/opt/trn_rl_repo" not in sys.path:
    sys.path.insert(0, "/opt/trn_rl_repo")

B, T, K = 1024, 1024, 32
NCORES = 8
BL = B // NCORES  # 128 batch rows per core
TC = 64  # time chunk (em streaming / tags8 chunking)
POS_BIG = 3.0e38

# forward j-split: DVE owns j in [0, JD), Pool owns j in [JD, K).
# NOTE: neuronxcc rejects generic compute (TensorTensor/TensorScalarPtr) on
# the Pool engine, so jpool must stay 0 on real hardware; the split code is
# kept for cost-model experiments only.
JPOOL = 0


def build_nc(t_steps: int = T, tc: int = TC, jpool: int = JPOOL):
    """Build + compile the per-core Bass program (same NEFF on all 8 cores)."""
    import concourse.bass as bass
    import concourse.tile as tile
    from concourse import bacc, mybir

    f32 = mybir.dt.float32
    u32 = mybir.dt.uint32
    i32 = mybir.dt.int32
    Alu = mybir.AluOpType
    Ax = mybir.AxisListType

    nsteps = t_steps
    nchunks = (nsteps + tc - 1) // tc
    assert nsteps % tc == 0

    nc = bacc.Bacc(
        "TRN2", target_bir_lowering=False, debug=False, enable_asserts=False
    )

    em_d = nc.dram_tensor("em", [BL, nsteps * K], f32, kind="ExternalInput").ap()
    ttb_d = nc.dram_tensor("ttb", [BL, K * K], f32, kind="ExternalInput").ap()
    tmov_d = nc.dram_tensor("tmov", [128, K], f32, kind="ExternalInput").ap()
    endt_d = nc.dram_tensor("endt", [BL, K], f32, kind="ExternalInput").ap()
    iota_d = nc.dram_tensor("iota", [BL, K], u32, kind="ExternalInput").ap()
    # block-diagonal iota: row-block r holds 0..31 at free offset 32*(r%2),
    # 255 elsewhere -> onehot64 is block-diagonal, so one 64-contraction
    # matmul covers two row blocks (2 matmuls/step instead of 4)
    iota64_d = nc.dram_tensor("iota64", [BL, 2 * K], u32, kind="ExternalInput").ap()
    tags_d = nc.dram_tensor("tags", [BL, nsteps], i32, kind="ExternalOutput").ap()

    with tile.TileContext(nc) as tc_ctx:
        _body(nc, tc_ctx, bass, mybir, Alu, Ax, f32, u32, i32,
              em_d, ttb_d, tmov_d, endt_d, iota_d, iota64_d, tags_d, nsteps,
              tc, nchunks, jpool)

    nc.compile()
    return nc


def _body(nc, tc_ctx, bass, mybir, Alu, Ax, f32, u32, i32,
          em_d, ttb_d, tmov_d, endt_d, iota_d, iota64_d, tags_d, nsteps,
          tc, nchunks, jpool):
    from contextlib import ExitStack

    from concourse.dve_ops import TENSOR_TENSOR_REDUCE as _CTTR

    jd = K - jpool  # DVE-owned j count

    ctx = ExitStack()
    with ctx:
        const_pool = ctx.enter_context(tc_ctx.tile_pool(name="const", bufs=1))
        hist_pool = ctx.enter_context(tc_ctx.tile_pool(name="hist", bufs=1))
        em_pool = ctx.enter_context(tc_ctx.tile_pool(name="em", bufs=2))
        work_pool = ctx.enter_context(tc_ctx.tile_pool(name="work", bufs=1))
        tags8_pool = ctx.enter_context(tc_ctx.tile_pool(name="tags8", bufs=2))
        psum_pool = ctx.enter_context(
            tc_ctx.tile_pool(name="psum", bufs=2, space="PSUM")
        )

        # ---- constants ----
        ttb = const_pool.tile([BL, K * K], f32)  # ttb[b, j*K+i] = trans[i, j]
        nc.sync.dma_start(ttb[:], ttb_d[:])
        tmov = const_pool.tile([128, K], f32)  # trans.T tiled x4 over partitions
        nc.sync.dma_start(tmov[:], tmov_d[:])
        endt = const_pool.tile([BL, K], f32)
        nc.sync.dma_start(endt[:], endt_d[:])
        iota = const_pool.tile([BL, K], u32)
        nc.sync.dma_start(iota[:], iota_d[:])
        iota64 = const_pool.tile([BL, 2 * K], u32)
        nc.sync.dma_start(iota64[:], iota64_d[:])

        # ---- working tiles ----
        hist = hist_pool.tile([BL, nsteps * K], f32)  # all forward scores
        cand = work_pool.tile([BL, max(jd, 1) * K], f32)  # DVE j-slice
        candp = work_pool.tile([BL, max(jpool, 1) * K], f32)  # Pool j-slice
        l1 = work_pool.tile([BL, max(jpool, 1) * (K // 2)], f32)
        premax = work_pool.tile([BL, K], f32)
        # double-buffered by step parity: breaks the Pool-write-after-DVE-read
        # serialization on the per-step scalar tiles
        m8s = [work_pool.tile([BL, 8], f32, name=f"m8_{i}") for i in range(2)]
        tmps = [work_pool.tile([BL, K], f32, name=f"tmp_{i}") for i in range(2)]
        emnegs = [work_pool.tile([BL, 1], f32, name=f"emneg_{i}") for i in range(2)]
        scrs = [work_pool.tile([BL, 2 * K], f32, name=f"scr_{i}") for i in range(2)]
        scr2s = [work_pool.tile([BL, 2 * K], f32, name=f"scr2_{i}") for i in range(2)]
        onehots = [work_pool.tile([BL, 2 * K], f32, name=f"oh_{i}") for i in range(2)]
        vts = [work_pool.tile([BL, 2 * K], f32, name=f"vt_{i}") for i in range(2)]
        tagout = work_pool.tile([BL, nsteps], i32)

        nc.vector.memset(m8s[0][:], POS_BIG)
        nc.vector.memset(m8s[1][:], POS_BIG)

        ttb3 = ttb[:].rearrange("p (j i) -> p j i", i=K)
        cand3 = cand[:].rearrange("p (j i) -> p j i", i=K)
        candp3 = candp[:].rearrange("p (j i) -> p j i", i=K)
        l13 = l1[:].rearrange("p (j i) -> p j i", i=K // 2)

        # ================= forward =================
        for c in range(nchunks):
            emf = em_pool.tile([BL, tc * K], f32, tag="emchunk")
            nc.sync.dma_start(emf[:], em_d[:, c * tc * K : (c + 1) * tc * K])
            for tloc in range(tc):
                t = c * tc + tloc
                em_sl = emf[:, tloc * K : (tloc + 1) * K]
                h_t = hist[:, t * K : (t + 1) * K]
                if t == 0:
                    nc.vector.tensor_copy(h_t, em_sl)
                    continue
                h_prev = hist[:, (t - 1) * K : t * K]
                h_bc = h_prev[:, None, :]
                if jpool > 0:
                    # Pool: cand slice [jd, K) + pairwise L1 max to 16-wide
                    nc.gpsimd.scalar_tensor_tensor(
                        candp3[:, 0:jpool, :],
                        h_bc.broadcast_to([BL, jpool, K]),
                        0.0,
                        ttb3[:, jd:K, :],
                        Alu.bypass,
                        Alu.add,
                    )
                    nc.gpsimd.tensor_tensor(
                        l13[:, 0:jpool, :],
                        candp3[:, 0:jpool, 0 : K // 2],
                        candp3[:, 0:jpool, K // 2 : K],
                        Alu.max,
                    )
                if jd > 0:
                    nc.vector.tensor_tensor(
                        cand3[:, 0:jd, :],
                        h_bc.broadcast_to([BL, jd, K]),
                        ttb3[:, 0:jd, :],
                        Alu.add,
                    )
                    nc.vector.tensor_reduce(
                        premax[:, 0:jd], cand3[:, 0:jd, :], Ax.X, Alu.max
                    )
                if jpool > 0:
                    nc.vector.tensor_reduce(
                        premax[:, jd:K], l13[:, 0:jpool, :], Ax.X, Alu.max
                    )
                nc.vector.tensor_tensor(h_t, premax[:], em_sl, Alu.add)

        # ================= final argmax =================
        # ref: score = hist[T-1] + end_transitions, then argmax (first index)
        tags8_cur = tags8_pool.tile([BL, tc * 8], u32, tag="t8")
        tmp0 = tmps[(nsteps - 1) % 2]
        m80 = m8s[(nsteps - 1) % 2]
        nc.vector.tensor_tensor(
            tmp0[:], hist[:, (nsteps - 1) * K : nsteps * K], endt[:], Alu.add
        )
        nc.vector.tensor_reduce(m80[:, 0:1], tmp0[:], Ax.X, Alu.max)
        last_slot = (nsteps - 1) - (nchunks - 1) * tc
        nc.vector.max_index(
            tags8_cur[:, last_slot * 8 : last_slot * 8 + 8], m80[:], tmp0[:]
        )

        # ================= backward =================
        tags8_by_chunk = {nchunks - 1: tags8_cur}
        for c in range(nchunks - 1, -1, -1):
            # em[s+1] for s in [c*tc, (c+1)*tc): dram slice offset by one step
            n_em = tc if c < nchunks - 1 else tc - 1
            embw = em_pool.tile([BL, tc * K], f32, tag="emchunk")
            nc.sync.dma_start(
                embw[:, : n_em * K],
                em_d[:, (c * tc + 1) * K : (c * tc + 1 + n_em) * K],
            )
            if c not in tags8_by_chunk:
                tags8_by_chunk[c] = tags8_pool.tile(
                    [BL, tc * 8], u32, tag="t8", name=f"t8c{c}"
                )
            t8c = tags8_by_chunk[c]

            s_hi = min(nsteps - 2, (c + 1) * tc - 1)
            for s in range(s_hi, c * tc - 1, -1):
                tloc = s - c * tc
                par = s % 2
                onehot, vt = onehots[par], vts[par]
                tmp, m8, emneg = tmps[par], m8s[par], emnegs[par]
                # one-hot of tag_{s+1}
                sp1 = s + 1
                cp1 = sp1 // tc
                t8p = tags8_by_chunk[cp1]
                slot = sp1 - cp1 * tc
                # block-diagonal one-hot of tag_{s+1} ([BL, 64]; row-block r
                # has its 32 valid lanes at free offset 32*(r%2), zeros
                # elsewhere since iota64 is 255 there)
                nc.vector.tensor_tensor(
                    onehot[:],
                    iota64[:],
                    t8p[:, slot * 8 : slot * 8 + 1].broadcast_to([BL, 2 * K]),
                    Alu.is_equal,
                )
                nc.vector.transpose(vt[:], onehot[:])
                # per-row scalars (DVE TTRs, run inside the PE window), using
                # the j-replicated view of em/hist so the 255-half contributes
                # exact zeros:
                # emneg[b] = -em_{s+1}[b, tag];
                # histsel[b] = hist_{s+1}[b, tag] -> m8 slot 0 (bitwise equal
                # to max(tmp), so the separate tensor_reduce is not needed, and
                # max_index has no cross-engine dep so it dispatches in-order)
                oh3 = onehot[:].rearrange("p (c i) -> p c i", i=K)
                em_sl_bw = embw[:, tloc * K : (tloc + 1) * K]
                nc.vector._custom_dve(
                    _CTTR,
                    out=scrs[par][:].rearrange("p (c i) -> p c i", i=K),
                    in0=oh3,
                    in1=em_sl_bw[:, None, :].broadcast_to([BL, 2, K]),
                    s0=0.0,
                    s1=-1.0,
                    accum_out=emneg[:],
                )
                nc.vector._custom_dve(
                    _CTTR,
                    out=scr2s[par][:].rearrange("p (c i) -> p c i", i=K),
                    in0=oh3,
                    in1=hist[:, sp1 * K : (sp1 + 1) * K][:, None, :]
                    .broadcast_to([BL, 2, K]),
                    s0=0.0,
                    s1=1.0,
                    accum_out=m8[:, 0:1],
                )
                # transsel[b,i] = trans[i, tag_b] via 2 diagonal 64x64 matmuls
                tsel = psum_pool.tile([BL, K], f32, tag="tsel")
                for r in range(2):
                    nc.tensor.matmul(
                        tsel[64 * r : 64 * r + 64, :],
                        vt[64 * r : 64 * r + 64, :],
                        tmov[64 * r : 64 * r + 64, :],
                        start=True,
                        stop=True,
                        tile_position=(64 * r, 64 * r),
                    )
                # tmp = (hist_s - tsel*(-1) - emneg)*1 = (hist_s+tsel)+em
                # -- bitwise identical to the ref association (sign flips and
                # subtract-of-negation are IEEE-exact).  The max value for
                # max_index is hist_{s+1}[tag] (bitwise == max(tmp)).
                nc.vector.ln_bwd_dx(
                    tmp[:], hist[:, s * K : (s + 1) * K], tsel[:], -1.0,
                    emneg[:], 1.0,
                )
                nc.vector.max_index(
                    t8c[:, tloc * 8 : tloc * 8 + 8], m8[:], tmp[:]
                )

            # compact this chunk's tags (slot stride 8 -> dense) on ScalarE
            t83 = t8c[:].rearrange("p (s e) -> p s e", e=8)
            nc.scalar.copy(
                tagout[:, c * tc : (c + 1) * tc][:, :, None], t83[:, :, 0:1]
            )
            nc.sync.dma_start(
                tags_d[:, c * tc : (c + 1) * tc], tagout[:, c * tc : (c + 1) * tc]
            )
            if c + 1 in tags8_by_chunk:
                del tags8_by_chunk[c + 1]


_NC_CACHE = {}


def _get_nc(t_steps=T, tc=TC, jpool=JPOOL):
    key = (t_steps, tc, jpool)
    if key not in _NC_CACHE:
        _NC_CACHE[key] = build_nc(t_steps, tc, jpool)
    return _NC_CACHE[key]


def make_in_maps(inputs, start_transitions, end_transitions, transitions,
                 t_steps=T):
    """Host-side shard + constant prep. Returns list of per-core input dicts."""
    inputs = np.asarray(inputs, np.float32)
    start = np.asarray(start_transitions, np.float32)
    end = np.asarray(end_transitions, np.float32)
    trans = np.asarray(transitions, np.float32)

    ttb = np.ascontiguousarray(
        np.broadcast_to(trans.T.reshape(1, K * K), (BL, K * K))
    )
    tmov = np.ascontiguousarray(np.tile(trans.T, (4, 1)))
    endt = np.ascontiguousarray(np.broadcast_to(end.reshape(1, K), (BL, K)))
    iota = np.ascontiguousarray(
        np.broadcast_to(np.arange(K, dtype=np.uint32), (BL, K))
    )
    # block-diagonal iota for the 64-wide onehot: row-block r holds 0..31 at
    # free offset 32*(r%2), 255 (never a tag) elsewhere
    iota64 = np.full((BL, 2 * K), 255, dtype=np.uint32)
    for r in range(BL // K):
        off = K * (r % 2)
        iota64[r * K : (r + 1) * K, off : off + K] = np.arange(K, dtype=np.uint32)

    in_maps = []
    for ci in range(NCORES):
        em = np.array(
            inputs[ci * BL : (ci + 1) * BL, :t_steps].reshape(BL, t_steps * K)
        )
        # fold start_transitions into em[0] (same association as the ref)
        em[:, :K] = start.reshape(1, K) + em[:, :K]
        in_maps.append(
            {"em": em, "ttb": ttb, "tmov": tmov, "endt": endt, "iota": iota,
             "iota64": iota64}
        )
    return in_maps


_last_result = None


def kernel(inputs, mask, start_transitions, end_transitions, transitions):
    global _last_result
    mask = np.asarray(mask)
    if not mask.all():
        return _numpy_fallback(
            np.asarray(inputs, np.float32), mask,
            np.asarray(start_transitions, np.float32),
            np.asarray(end_transitions, np.float32),
            np.asarray(transitions, np.float32),
        )

    from concourse.bass_utils import run_bass_kernel_spmd

    nc = _get_nc()
    in_maps = make_in_maps(inputs, start_transitions, end_transitions, transitions)
    res = run_bass_kernel_spmd(nc, in_maps, core_ids=list(range(NCORES)))
    _last_result = res
    tags = np.concatenate([res.results[i]["tags"] for i in range(NCORES)], axis=0)
    return tags.astype(np.int32)


def _numpy_fallback(inputs, mask, start, end, trans):
    """Vectorized numpy Viterbi matching torchcrf/ref semantics (general mask)."""
    em = np.swapaxes(inputs, 0, 1)  # [T, B, K]
    mk = np.swapaxes(mask, 0, 1)  # [T, B]
    nT, nB, nK = em.shape
    score = start[None, :] + em[0]
    hist = np.zeros((nT - 1, nB, nK), np.int32)
    for t in range(1, nT):
        cand = score[:, :, None] + trans[None, :, :] + em[t][:, None, :]
        bp = np.argmax(cand, axis=1).astype(np.int32)
        ns = np.max(cand, axis=1)
        m = mk[t][:, None]
        score = np.where(m, ns, score)
        hist[t - 1] = bp
    score = score + end[None, :]
    tag = np.argmax(score, axis=1).astype(np.int32)
    tags = np.zeros((nT, nB), np.int32)
    tags[nT - 1] = tag
    for t in range(nT - 2, -1, -1):
        prev = np.take_along_axis(hist[t], tag[:, None], axis=1)[:, 0]
        prev = np.where(mk[t + 1], prev, tag)
        tags[t] = prev
        tag = prev
    return np.swapaxes(tags, 0, 1).astype(np.int32)


# revision 22
# speedup vs baseline: 1.0010x; 1.0005x over previous
"""CRF Viterbi decode (torchcrf semantics) on 8 Trainium2 NeuronCores.

Strategy: pure data parallel over batch (1024 rows -> 128 rows/core, one row
per SBUF partition).  Per core:

  Forward (DVE only, scores resident in SBUF, bit-exact vs the jax ref):
    cand[b,j,i] = score[b,i] + trans[i,j]   (stride-0 broadcast TT add)
    premax[b,j] = max_i cand[b,j,i]         (segmented tensor_reduce)
    score'[b,j] = premax[b,j] + em[b,t,j]   (small TT add)
  This 3-op chain is the DVE floor: neuronxcc rejects generic compute on the
  Pool engine, the Activation engine only takes [P,1] bias vectors, fp32 PE
  matmuls run at 4 cycles/row, and no DVE op fuses a tensor-tensor stage
  with a segmented reduce.  Any further op-splitting loses to the ~95ns
  per-op overhead plus the full-vector barrier each step carries.

  Backward (recomputes each step's candidates instead of storing bp):
    A 64-wide block-diagonal one-hot of tag_{s+1} (iota64 holds 0..31 at
    free offset 32*(r%2) for row-block r, 255 elsewhere) is block-transposed
    by the 32x32 vector-transpose, so TWO 64-contraction tile_position
    matmuls gather transsel[b,i] = trans[i, tag_{s+1}(b)] (vs four 32-wide).
    The max value needed by max_index is not recomputed: it equals
    hist_{s+1}[b, tag_{s+1}] bitwise (max-then-add-const == add-const-then-
    max for fp max VALUES), so the per-row scalars are gathered by two DVE
    TTR ops that run inside the PE window:
      emneg[b]  = sum (onehot * em_{s+1-replicated}) * -1
      histsel[b] = sum (onehot * hist_{s+1-replicated}) -> m8 slot 0
    tmp = (hist_s - transsel*(-1) - emneg)*1  (ln_bwd_dx; associations match
    the ref exactly), then max_index against histsel (first-index tie break
    = jnp.argmax).  All max_index inputs are same-engine, so it dispatches
    in-order with no cross-engine event-semaphore wait.

Inputs are taken at full shape; sharding/gather happens on host inside
kernel().
"""

import sys

import numpy as np

if "/opt/trn_rl_repo" not in sys.path:
    sys.path.insert(0, "/opt/trn_rl_repo")

B, T, K = 1024, 1024, 32
NCORES = 8
BL = B // NCORES  # 128 batch rows per core
TC = 64  # time chunk (em streaming / tags8 chunking)
POS_BIG = 3.0e38

# forward j-split: DVE owns j in [0, JD), Pool owns j in [JD, K).
# NOTE: neuronxcc rejects generic compute (TensorTensor/TensorScalarPtr) on
# the Pool engine, so jpool must stay 0 on real hardware; the split code is
# kept for cost-model experiments only.
JPOOL = 0


def build_nc(t_steps: int = T, tc: int = TC, jpool: int = JPOOL):
    """Build + compile the per-core Bass program (same NEFF on all 8 cores)."""
    import concourse.bass as bass
    import concourse.tile as tile
    from concourse import bacc, mybir

    f32 = mybir.dt.float32
    u32 = mybir.dt.uint32
    i32 = mybir.dt.int32
    Alu = mybir.AluOpType
    Ax = mybir.AxisListType

    nsteps = t_steps
    nchunks = (nsteps + tc - 1) // tc
    assert nsteps % tc == 0

    nc = bacc.Bacc(
        "TRN2", target_bir_lowering=False, debug=False, enable_asserts=False
    )

    em_d = nc.dram_tensor("em", [BL, nsteps * K], f32, kind="ExternalInput").ap()
    ttb_d = nc.dram_tensor("ttb", [BL, K * K], f32, kind="ExternalInput").ap()
    tmov_d = nc.dram_tensor("tmov", [128, K], f32, kind="ExternalInput").ap()
    endt_d = nc.dram_tensor("endt", [BL, K], f32, kind="ExternalInput").ap()
    iota_d = nc.dram_tensor("iota", [BL, K], u32, kind="ExternalInput").ap()
    # block-diagonal iota: row-block r holds 0..31 at free offset 32*(r%2),
    # 255 elsewhere -> onehot64 is block-diagonal, so one 64-contraction
    # matmul covers two row blocks (2 matmuls/step instead of 4)
    iota64_d = nc.dram_tensor("iota64", [BL, 2 * K], u32, kind="ExternalInput").ap()
    tags_d = nc.dram_tensor("tags", [BL, nsteps], i32, kind="ExternalOutput").ap()

    with tile.TileContext(nc) as tc_ctx:
        _body(nc, tc_ctx, bass, mybir, Alu, Ax, f32, u32, i32,
              em_d, ttb_d, tmov_d, endt_d, iota_d, iota64_d, tags_d, nsteps,
              tc, nchunks, jpool)

    nc.compile()
    return nc


def _body(nc, tc_ctx, bass, mybir, Alu, Ax, f32, u32, i32,
          em_d, ttb_d, tmov_d, endt_d, iota_d, iota64_d, tags_d, nsteps,
          tc, nchunks, jpool):
    from contextlib import ExitStack

    from concourse.dve_ops import TENSOR_TENSOR_REDUCE as _CTTR

    jd = K - jpool  # DVE-owned j count

    ctx = ExitStack()
    with ctx:
        const_pool = ctx.enter_context(tc_ctx.tile_pool(name="const", bufs=1))
        hist_pool = ctx.enter_context(tc_ctx.tile_pool(name="hist", bufs=1))
        em_pool = ctx.enter_context(tc_ctx.tile_pool(name="em", bufs=2))
        work_pool = ctx.enter_context(tc_ctx.tile_pool(name="work", bufs=1))
        tags8_pool = ctx.enter_context(tc_ctx.tile_pool(name="tags8", bufs=2))
        psum_pool = ctx.enter_context(
            tc_ctx.tile_pool(name="psum", bufs=2, space="PSUM")
        )

        # ---- constants ----
        # em chunk 0 is what step t=0 needs first, then ttb for t=1; the rest
        # (tmov/endt/iota*) is only read ~3ms later by the backward, so issue
        # the forward-critical transfers first.
        emf0 = em_pool.tile([BL, tc * K], f32, tag="emchunk")
        nc.sync.dma_start(emf0[:], em_d[:, 0 : tc * K])
        ttb = const_pool.tile([BL, K * K], f32)  # ttb[b, j*K+i] = trans[i, j]
        nc.sync.dma_start(ttb[:], ttb_d[:])
        tmov = const_pool.tile([128, K], f32)  # trans.T tiled x4 over partitions
        nc.sync.dma_start(tmov[:], tmov_d[:])
        endt = const_pool.tile([BL, K], f32)
        nc.sync.dma_start(endt[:], endt_d[:])
        iota = const_pool.tile([BL, K], u32)
        nc.sync.dma_start(iota[:], iota_d[:])
        iota64 = const_pool.tile([BL, 2 * K], u32)
        nc.sync.dma_start(iota64[:], iota64_d[:])

        # ---- working tiles ----
        hist = hist_pool.tile([BL, nsteps * K], f32)  # all forward scores
        cand = work_pool.tile([BL, max(jd, 1) * K], f32)  # DVE j-slice
        candp = work_pool.tile([BL, max(jpool, 1) * K], f32)  # Pool j-slice
        l1 = work_pool.tile([BL, max(jpool, 1) * (K // 2)], f32)
        premax = work_pool.tile([BL, K], f32)
        # double-buffered by step parity: breaks the Pool-write-after-DVE-read
        # serialization on the per-step scalar tiles
        m8s = [work_pool.tile([BL, 8], f32, name=f"m8_{i}") for i in range(2)]
        tmps = [work_pool.tile([BL, K], f32, name=f"tmp_{i}") for i in range(2)]
        emnegs = [work_pool.tile([BL, 1], f32, name=f"emneg_{i}") for i in range(2)]
        scrs = [work_pool.tile([BL, 2 * K], f32, name=f"scr_{i}") for i in range(2)]
        scr2s = [work_pool.tile([BL, 2 * K], f32, name=f"scr2_{i}") for i in range(2)]
        onehots = [work_pool.tile([BL, 2 * K], f32, name=f"oh_{i}") for i in range(2)]
        vts = [work_pool.tile([BL, 2 * K], f32, name=f"vt_{i}") for i in range(2)]
        tagout = work_pool.tile([BL, nsteps], i32)

        nc.vector.memset(m8s[0][:], POS_BIG)
        nc.vector.memset(m8s[1][:], POS_BIG)

        ttb3 = ttb[:].rearrange("p (j i) -> p j i", i=K)
        cand3 = cand[:].rearrange("p (j i) -> p j i", i=K)
        candp3 = candp[:].rearrange("p (j i) -> p j i", i=K)
        l13 = l1[:].rearrange("p (j i) -> p j i", i=K // 2)

        # ================= forward =================
        for c in range(nchunks):
            if c == 0:
                emf = emf0
            else:
                emf = em_pool.tile([BL, tc * K], f32, tag="emchunk")
                nc.sync.dma_start(emf[:], em_d[:, c * tc * K : (c + 1) * tc * K])
            for tloc in range(tc):
                t = c * tc + tloc
                em_sl = emf[:, tloc * K : (tloc + 1) * K]
                h_t = hist[:, t * K : (t + 1) * K]
                if t == 0:
                    nc.vector.tensor_copy(h_t, em_sl)
                    continue
                h_prev = hist[:, (t - 1) * K : t * K]
                h_bc = h_prev[:, None, :]
                if jpool > 0:
                    # Pool: cand slice [jd, K) + pairwise L1 max to 16-wide
                    nc.gpsimd.scalar_tensor_tensor(
                        candp3[:, 0:jpool, :],
                        h_bc.broadcast_to([BL, jpool, K]),
                        0.0,
                        ttb3[:, jd:K, :],
                        Alu.bypass,
                        Alu.add,
                    )
                    nc.gpsimd.tensor_tensor(
                        l13[:, 0:jpool, :],
                        candp3[:, 0:jpool, 0 : K // 2],
                        candp3[:, 0:jpool, K // 2 : K],
                        Alu.max,
                    )
                if jd > 0:
                    nc.vector.tensor_tensor(
                        cand3[:, 0:jd, :],
                        h_bc.broadcast_to([BL, jd, K]),
                        ttb3[:, 0:jd, :],
                        Alu.add,
                    )
                    nc.vector.tensor_reduce(
                        premax[:, 0:jd], cand3[:, 0:jd, :], Ax.X, Alu.max
                    )
                if jpool > 0:
                    nc.vector.tensor_reduce(
                        premax[:, jd:K], l13[:, 0:jpool, :], Ax.X, Alu.max
                    )
                nc.vector.tensor_tensor(h_t, premax[:], em_sl, Alu.add)

        # ================= final argmax =================
        # ref: score = hist[T-1] + end_transitions, then argmax (first index)
        tags8_cur = tags8_pool.tile([BL, tc * 8], u32, tag="t8")
        tmp0 = tmps[(nsteps - 1) % 2]
        m80 = m8s[(nsteps - 1) % 2]
        nc.vector.tensor_tensor(
            tmp0[:], hist[:, (nsteps - 1) * K : nsteps * K], endt[:], Alu.add
        )
        nc.vector.tensor_reduce(m80[:, 0:1], tmp0[:], Ax.X, Alu.max)
        last_slot = (nsteps - 1) - (nchunks - 1) * tc
        nc.vector.max_index(
            tags8_cur[:, last_slot * 8 : last_slot * 8 + 8], m80[:], tmp0[:]
        )

        # ================= backward =================
        tags8_by_chunk = {nchunks - 1: tags8_cur}
        for c in range(nchunks - 1, -1, -1):
            # em[s+1] for s in [c*tc, (c+1)*tc): dram slice offset by one step
            n_em = tc if c < nchunks - 1 else tc - 1
            embw = em_pool.tile([BL, tc * K], f32, tag="emchunk")
            nc.sync.dma_start(
                embw[:, : n_em * K],
                em_d[:, (c * tc + 1) * K : (c * tc + 1 + n_em) * K],
            )
            if c not in tags8_by_chunk:
                tags8_by_chunk[c] = tags8_pool.tile(
                    [BL, tc * 8], u32, tag="t8", name=f"t8c{c}"
                )
            t8c = tags8_by_chunk[c]

            s_hi = min(nsteps - 2, (c + 1) * tc - 1)
            for s in range(s_hi, c * tc - 1, -1):
                tloc = s - c * tc
                par = s % 2
                onehot, vt = onehots[par], vts[par]
                tmp, m8, emneg = tmps[par], m8s[par], emnegs[par]
                # one-hot of tag_{s+1}
                sp1 = s + 1
                cp1 = sp1 // tc
                t8p = tags8_by_chunk[cp1]
                slot = sp1 - cp1 * tc
                # block-diagonal one-hot of tag_{s+1} ([BL, 64]; row-block r
                # has its 32 valid lanes at free offset 32*(r%2), zeros
                # elsewhere since iota64 is 255 there)
                nc.vector.tensor_tensor(
                    onehot[:],
                    iota64[:],
                    t8p[:, slot * 8 : slot * 8 + 1].broadcast_to([BL, 2 * K]),
                    Alu.is_equal,
                )
                nc.vector.transpose(vt[:], onehot[:])
                # per-row scalars (DVE TTRs, run inside the PE window), using
                # the j-replicated view of em/hist so the 255-half contributes
                # exact zeros:
                # emneg[b] = -em_{s+1}[b, tag];
                # histsel[b] = hist_{s+1}[b, tag] -> m8 slot 0 (bitwise equal
                # to max(tmp), so the separate tensor_reduce is not needed, and
                # max_index has no cross-engine dep so it dispatches in-order)
                oh3 = onehot[:].rearrange("p (c i) -> p c i", i=K)
                em_sl_bw = embw[:, tloc * K : (tloc + 1) * K]
                nc.vector._custom_dve(
                    _CTTR,
                    out=scrs[par][:].rearrange("p (c i) -> p c i", i=K),
                    in0=oh3,
                    in1=em_sl_bw[:, None, :].broadcast_to([BL, 2, K]),
                    s0=0.0,
                    s1=-1.0,
                    accum_out=emneg[:],
                )
                nc.vector._custom_dve(
                    _CTTR,
                    out=scr2s[par][:].rearrange("p (c i) -> p c i", i=K),
                    in0=oh3,
                    in1=hist[:, sp1 * K : (sp1 + 1) * K][:, None, :]
                    .broadcast_to([BL, 2, K]),
                    s0=0.0,
                    s1=1.0,
                    accum_out=m8[:, 0:1],
                )
                # transsel[b,i] = trans[i, tag_b] via 2 diagonal 64x64 matmuls
                tsel = psum_pool.tile([BL, K], f32, tag="tsel")
                for r in range(2):
                    nc.tensor.matmul(
                        tsel[64 * r : 64 * r + 64, :],
                        vt[64 * r : 64 * r + 64, :],
                        tmov[64 * r : 64 * r + 64, :],
                        start=True,
                        stop=True,
                        tile_position=(64 * r, 64 * r),
                    )
                # tmp = (hist_s - tsel*(-1) - emneg)*1 = (hist_s+tsel)+em
                # -- bitwise identical to the ref association (sign flips and
                # subtract-of-negation are IEEE-exact).  The max value for
                # max_index is hist_{s+1}[tag] (bitwise == max(tmp)).
                nc.vector.ln_bwd_dx(
                    tmp[:], hist[:, s * K : (s + 1) * K], tsel[:], -1.0,
                    emneg[:], 1.0,
                )
                nc.vector.max_index(
                    t8c[:, tloc * 8 : tloc * 8 + 8], m8[:], tmp[:]
                )

            # compact this chunk's tags (slot stride 8 -> dense) on ScalarE
            t83 = t8c[:].rearrange("p (s e) -> p s e", e=8)
            nc.scalar.copy(
                tagout[:, c * tc : (c + 1) * tc][:, :, None], t83[:, :, 0:1]
            )
            nc.sync.dma_start(
                tags_d[:, c * tc : (c + 1) * tc], tagout[:, c * tc : (c + 1) * tc]
            )
            if c + 1 in tags8_by_chunk:
                del tags8_by_chunk[c + 1]


_NC_CACHE = {}


def _get_nc(t_steps=T, tc=TC, jpool=JPOOL):
    key = (t_steps, tc, jpool)
    if key not in _NC_CACHE:
        _NC_CACHE[key] = build_nc(t_steps, tc, jpool)
    return _NC_CACHE[key]


def make_in_maps(inputs, start_transitions, end_transitions, transitions,
                 t_steps=T):
    """Host-side shard + constant prep. Returns list of per-core input dicts."""
    inputs = np.asarray(inputs, np.float32)
    start = np.asarray(start_transitions, np.float32)
    end = np.asarray(end_transitions, np.float32)
    trans = np.asarray(transitions, np.float32)

    ttb = np.ascontiguousarray(
        np.broadcast_to(trans.T.reshape(1, K * K), (BL, K * K))
    )
    tmov = np.ascontiguousarray(np.tile(trans.T, (4, 1)))
    endt = np.ascontiguousarray(np.broadcast_to(end.reshape(1, K), (BL, K)))
    iota = np.ascontiguousarray(
        np.broadcast_to(np.arange(K, dtype=np.uint32), (BL, K))
    )
    # block-diagonal iota for the 64-wide onehot: row-block r holds 0..31 at
    # free offset 32*(r%2), 255 (never a tag) elsewhere
    iota64 = np.full((BL, 2 * K), 255, dtype=np.uint32)
    for r in range(BL // K):
        off = K * (r % 2)
        iota64[r * K : (r + 1) * K, off : off + K] = np.arange(K, dtype=np.uint32)

    in_maps = []
    for ci in range(NCORES):
        em = np.array(
            inputs[ci * BL : (ci + 1) * BL, :t_steps].reshape(BL, t_steps * K)
        )
        # fold start_transitions into em[0] (same association as the ref)
        em[:, :K] = start.reshape(1, K) + em[:, :K]
        in_maps.append(
            {"em": em, "ttb": ttb, "tmov": tmov, "endt": endt, "iota": iota,
             "iota64": iota64}
        )
    return in_maps


_last_result = None


def kernel(inputs, mask, start_transitions, end_transitions, transitions):
    global _last_result
    mask = np.asarray(mask)
    if not mask.all():
        return _numpy_fallback(
            np.asarray(inputs, np.float32), mask,
            np.asarray(start_transitions, np.float32),
            np.asarray(end_transitions, np.float32),
            np.asarray(transitions, np.float32),
        )

    from concourse.bass_utils import run_bass_kernel_spmd

    nc = _get_nc()
    in_maps = make_in_maps(inputs, start_transitions, end_transitions, transitions)
    res = run_bass_kernel_spmd(nc, in_maps, core_ids=list(range(NCORES)))
    _last_result = res
    tags = np.concatenate([res.results[i]["tags"] for i in range(NCORES)], axis=0)
    return tags.astype(np.int32)


def _numpy_fallback(inputs, mask, start, end, trans):
    """Vectorized numpy Viterbi matching torchcrf/ref semantics (general mask)."""
    em = np.swapaxes(inputs, 0, 1)  # [T, B, K]
    mk = np.swapaxes(mask, 0, 1)  # [T, B]
    nT, nB, nK = em.shape
    score = start[None, :] + em[0]
    hist = np.zeros((nT - 1, nB, nK), np.int32)
    for t in range(1, nT):
        cand = score[:, :, None] + trans[None, :, :] + em[t][:, None, :]
        bp = np.argmax(cand, axis=1).astype(np.int32)
        ns = np.max(cand, axis=1)
        m = mk[t][:, None]
        score = np.where(m, ns, score)
        hist[t - 1] = bp
    score = score + end[None, :]
    tag = np.argmax(score, axis=1).astype(np.int32)
    tags = np.zeros((nT, nB), np.int32)
    tags[nT - 1] = tag
    for t in range(nT - 2, -1, -1):
        prev = np.take_along_axis(hist[t], tag[:, None], axis=1)[:, 0]
        prev = np.where(mk[t + 1], prev, tag)
        tags[t] = prev
        tag = prev
    return np.swapaxes(tags, 0, 1).astype(np.int32)


# revision 25
# speedup vs baseline: 1.0010x; 1.0000x over previous
"""CRF Viterbi decode (torchcrf semantics) on 8 Trainium2 NeuronCores.

Strategy: pure data parallel over batch (1024 rows -> 128 rows/core, one row
per SBUF partition).  Per core:

  Forward (DVE only, scores resident in SBUF, bit-exact vs the jax ref):
    cand[b,j,i] = score[b,i] + trans[i,j]   (stride-0 broadcast TT add)
    premax[b,j] = max_i cand[b,j,i]         (segmented tensor_reduce)
    score'[b,j] = premax[b,j] + em[b,t,j]   (small TT add)
  This 3-op chain is the DVE floor: neuronxcc rejects generic compute on the
  Pool engine, the Activation engine only takes [P,1] bias vectors, fp32 PE
  matmuls run at 4 cycles/row, and no DVE op fuses a tensor-tensor stage
  with a segmented reduce.  Any further op-splitting loses to the ~95ns
  per-op overhead plus the full-vector barrier each step carries.

  Backward (recomputes each step's candidates instead of storing bp):
    A 64-wide block-diagonal one-hot of tag_{s+1} (iota64 holds 0..31 at
    free offset 32*(r%2) for row-block r, 255 elsewhere) is block-transposed
    by the 32x32 vector-transpose, so TWO 64-contraction tile_position
    matmuls gather transsel[b,i] = trans[i, tag_{s+1}(b)] (vs four 32-wide).
    The max value needed by max_index is not recomputed: it equals
    hist_{s+1}[b, tag_{s+1}] bitwise (max-then-add-const == add-const-then-
    max for fp max VALUES), so the per-row scalars are gathered by two DVE
    TTR ops that run inside the PE window:
      emneg[b]  = sum (onehot * em_{s+1-replicated}) * -1
      histsel[b] = sum (onehot * hist_{s+1-replicated}) -> m8 slot 0
    tmp = (hist_s - transsel*(-1) - emneg)*1  (ln_bwd_dx; associations match
    the ref exactly), then max_index against histsel (first-index tie break
    = jnp.argmax).  All max_index inputs are same-engine, so it dispatches
    in-order with no cross-engine event-semaphore wait.

Inputs are taken at full shape; sharding/gather happens on host inside
kernel().
"""

import sys

import numpy as np

if "/opt/trn_rl_repo" not in sys.path:
    sys.path.insert(0, "/opt/trn_rl_repo")

B, T, K = 1024, 1024, 32
NCORES = 8
BL = B // NCORES  # 128 batch rows per core
TC = 64  # time chunk (em streaming / tags8 chunking)
POS_BIG = 3.0e38

# forward j-split: DVE owns j in [0, JD), Pool owns j in [JD, K).
# NOTE: neuronxcc rejects generic compute (TensorTensor/TensorScalarPtr) on
# the Pool engine, so jpool must stay 0 on real hardware; the split code is
# kept for cost-model experiments only.
JPOOL = 0


def build_nc(t_steps: int = T, tc: int = TC, jpool: int = JPOOL):
    """Build + compile the per-core Bass program (same NEFF on all 8 cores)."""
    import concourse.bass as bass
    import concourse.tile as tile
    from concourse import bacc, mybir

    f32 = mybir.dt.float32
    u32 = mybir.dt.uint32
    i32 = mybir.dt.int32
    Alu = mybir.AluOpType
    Ax = mybir.AxisListType

    nsteps = t_steps
    nchunks = (nsteps + tc - 1) // tc
    assert nsteps % tc == 0

    nc = bacc.Bacc(
        "TRN2", target_bir_lowering=False, debug=False, enable_asserts=False
    )

    em_d = nc.dram_tensor("em", [BL, nsteps * K], f32, kind="ExternalInput").ap()
    ttb_d = nc.dram_tensor("ttb", [BL, K * K], f32, kind="ExternalInput").ap()
    tmov_d = nc.dram_tensor("tmov", [128, K], f32, kind="ExternalInput").ap()
    endt_d = nc.dram_tensor("endt", [BL, K], f32, kind="ExternalInput").ap()
    iota_d = nc.dram_tensor("iota", [BL, K], u32, kind="ExternalInput").ap()
    # block-diagonal iota: row-block r holds 0..31 at free offset 32*(r%2),
    # 255 elsewhere -> onehot64 is block-diagonal, so one 64-contraction
    # matmul covers two row blocks (2 matmuls/step instead of 4)
    iota64_d = nc.dram_tensor("iota64", [BL, 2 * K], u32, kind="ExternalInput").ap()
    tags_d = nc.dram_tensor("tags", [BL, nsteps], i32, kind="ExternalOutput").ap()

    with tile.TileContext(nc) as tc_ctx:
        _body(nc, tc_ctx, bass, mybir, Alu, Ax, f32, u32, i32,
              em_d, ttb_d, tmov_d, endt_d, iota_d, iota64_d, tags_d, nsteps,
              tc, nchunks, jpool)

    nc.compile()
    return nc


def _body(nc, tc_ctx, bass, mybir, Alu, Ax, f32, u32, i32,
          em_d, ttb_d, tmov_d, endt_d, iota_d, iota64_d, tags_d, nsteps,
          tc, nchunks, jpool):
    from contextlib import ExitStack

    from concourse.dve_ops import TENSOR_TENSOR_REDUCE as _CTTR

    jd = K - jpool  # DVE-owned j count

    ctx = ExitStack()
    with ctx:
        const_pool = ctx.enter_context(tc_ctx.tile_pool(name="const", bufs=1))
        hist_pool = ctx.enter_context(tc_ctx.tile_pool(name="hist", bufs=1))
        em_pool = ctx.enter_context(tc_ctx.tile_pool(name="em", bufs=2))
        work_pool = ctx.enter_context(tc_ctx.tile_pool(name="work", bufs=1))
        tags8_pool = ctx.enter_context(tc_ctx.tile_pool(name="tags8", bufs=2))
        psum_pool = ctx.enter_context(
            tc_ctx.tile_pool(name="psum", bufs=2, space="PSUM")
        )

        # ---- constants ----
        # em chunk 0 is what step t=0 needs first, then ttb for t=1; the rest
        # (tmov/endt/iota*) is only read ~3ms later by the backward, so issue
        # the forward-critical transfers first.
        emf0 = em_pool.tile([BL, tc * K], f32, tag="emchunk")
        nc.sync.dma_start(emf0[:], em_d[:, 0 : tc * K])
        ttb = const_pool.tile([BL, K * K], f32)  # ttb[b, j*K+i] = trans[i, j]
        nc.sync.dma_start(ttb[:], ttb_d[:])
        tmov = const_pool.tile([128, K], f32)  # trans.T tiled x4 over partitions
        nc.sync.dma_start(tmov[:], tmov_d[:])
        endt = const_pool.tile([BL, K], f32)
        nc.sync.dma_start(endt[:], endt_d[:])
        iota = const_pool.tile([BL, K], u32)
        nc.sync.dma_start(iota[:], iota_d[:])
        iota64 = const_pool.tile([BL, 2 * K], u32)
        nc.sync.dma_start(iota64[:], iota64_d[:])

        # ---- working tiles ----
        hist = hist_pool.tile([BL, nsteps * K], f32)  # all forward scores
        cand = work_pool.tile([BL, max(jd, 1) * K], f32)  # DVE j-slice
        candp = work_pool.tile([BL, max(jpool, 1) * K], f32)  # Pool j-slice
        l1 = work_pool.tile([BL, max(jpool, 1) * (K // 2)], f32)
        premax = work_pool.tile([BL, K], f32)
        # double-buffered by step parity: breaks the Pool-write-after-DVE-read
        # serialization on the per-step scalar tiles
        m8s = [work_pool.tile([BL, 8], f32, name=f"m8_{i}") for i in range(2)]
        tmps = [work_pool.tile([BL, K], f32, name=f"tmp_{i}") for i in range(2)]
        emnegs = [work_pool.tile([BL, 1], f32, name=f"emneg_{i}") for i in range(2)]
        scrs = [work_pool.tile([BL, 2 * K], f32, name=f"scr_{i}") for i in range(2)]
        scr2s = [work_pool.tile([BL, 2 * K], f32, name=f"scr2_{i}") for i in range(2)]
        onehots = [work_pool.tile([BL, 2 * K], f32, name=f"oh_{i}") for i in range(2)]
        vts = [work_pool.tile([BL, 2 * K], f32, name=f"vt_{i}") for i in range(2)]
        tagout = work_pool.tile([BL, nsteps], i32)

        nc.vector.memset(m8s[0][:], POS_BIG)
        nc.vector.memset(m8s[1][:], POS_BIG)

        ttb3 = ttb[:].rearrange("p (j i) -> p j i", i=K)
        cand3 = cand[:].rearrange("p (j i) -> p j i", i=K)
        candp3 = candp[:].rearrange("p (j i) -> p j i", i=K)
        l13 = l1[:].rearrange("p (j i) -> p j i", i=K // 2)

        # ================= forward =================
        for c in range(nchunks):
            if c == 0:
                emf = emf0
            else:
                emf = em_pool.tile([BL, tc * K], f32, tag="emchunk")
                nc.sync.dma_start(emf[:], em_d[:, c * tc * K : (c + 1) * tc * K])
            for tloc in range(tc):
                t = c * tc + tloc
                em_sl = emf[:, tloc * K : (tloc + 1) * K]
                h_t = hist[:, t * K : (t + 1) * K]
                if t == 0:
                    nc.vector.tensor_copy(h_t, em_sl)
                    continue
                h_prev = hist[:, (t - 1) * K : t * K]
                h_bc = h_prev[:, None, :]
                if jpool > 0:
                    # Pool: cand slice [jd, K) + pairwise L1 max to 16-wide
                    nc.gpsimd.scalar_tensor_tensor(
                        candp3[:, 0:jpool, :],
                        h_bc.broadcast_to([BL, jpool, K]),
                        0.0,
                        ttb3[:, jd:K, :],
                        Alu.bypass,
                        Alu.add,
                    )
                    nc.gpsimd.tensor_tensor(
                        l13[:, 0:jpool, :],
                        candp3[:, 0:jpool, 0 : K // 2],
                        candp3[:, 0:jpool, K // 2 : K],
                        Alu.max,
                    )
                if jd > 0:
                    nc.vector.tensor_tensor(
                        cand3[:, 0:jd, :],
                        h_bc.broadcast_to([BL, jd, K]),
                        ttb3[:, 0:jd, :],
                        Alu.add,
                    )
                    nc.vector.tensor_reduce(
                        premax[:, 0:jd], cand3[:, 0:jd, :], Ax.X, Alu.max
                    )
                if jpool > 0:
                    nc.vector.tensor_reduce(
                        premax[:, jd:K], l13[:, 0:jpool, :], Ax.X, Alu.max
                    )
                nc.vector.tensor_tensor(h_t, premax[:], em_sl, Alu.add)

        # ================= final argmax =================
        # ref: score = hist[T-1] + end_transitions, then argmax (first index)
        tags8_cur = tags8_pool.tile([BL, tc * 8], u32, tag="t8")
        tmp0 = tmps[(nsteps - 1) % 2]
        m80 = m8s[(nsteps - 1) % 2]
        nc.vector.tensor_tensor(
            tmp0[:], hist[:, (nsteps - 1) * K : nsteps * K], endt[:], Alu.add
        )
        nc.vector.tensor_reduce(m80[:, 0:1], tmp0[:], Ax.X, Alu.max)
        last_slot = (nsteps - 1) - (nchunks - 1) * tc
        nc.vector.max_index(
            tags8_cur[:, last_slot * 8 : last_slot * 8 + 8], m80[:], tmp0[:]
        )

        # ================= backward =================
        tags8_by_chunk = {nchunks - 1: tags8_cur}
        for c in range(nchunks - 1, -1, -1):
            # em[s+1] for s in [c*tc, (c+1)*tc): dram slice offset by one step
            n_em = tc if c < nchunks - 1 else tc - 1
            embw = em_pool.tile([BL, tc * K], f32, tag="emchunk")
            nc.sync.dma_start(
                embw[:, : n_em * K],
                em_d[:, (c * tc + 1) * K : (c * tc + 1 + n_em) * K],
            )
            if c not in tags8_by_chunk:
                tags8_by_chunk[c] = tags8_pool.tile(
                    [BL, tc * 8], u32, tag="t8", name=f"t8c{c}"
                )
            t8c = tags8_by_chunk[c]

            s_hi = min(nsteps - 2, (c + 1) * tc - 1)
            for s in range(s_hi, c * tc - 1, -1):
                tloc = s - c * tc
                par = s % 2
                onehot, vt = onehots[par], vts[par]
                tmp, m8, emneg = tmps[par], m8s[par], emnegs[par]
                # one-hot of tag_{s+1}
                sp1 = s + 1
                cp1 = sp1 // tc
                t8p = tags8_by_chunk[cp1]
                slot = sp1 - cp1 * tc
                # block-diagonal one-hot of tag_{s+1} ([BL, 64]; row-block r
                # has its 32 valid lanes at free offset 32*(r%2), zeros
                # elsewhere since iota64 is 255 there)
                nc.vector.tensor_tensor(
                    onehot[:],
                    iota64[:],
                    t8p[:, slot * 8 : slot * 8 + 1].broadcast_to([BL, 2 * K]),
                    Alu.is_equal,
                )
                nc.vector.transpose(vt[:], onehot[:])
                # per-row scalars (DVE TTRs, run inside the PE window), using
                # the j-replicated view of em/hist so the 255-half contributes
                # exact zeros:
                # emneg[b] = -em_{s+1}[b, tag];
                # histsel[b] = hist_{s+1}[b, tag] -> m8 slot 0 (bitwise equal
                # to max(tmp), so the separate tensor_reduce is not needed, and
                # max_index has no cross-engine dep so it dispatches in-order)
                oh3 = onehot[:].rearrange("p (c i) -> p c i", i=K)
                em_sl_bw = embw[:, tloc * K : (tloc + 1) * K]
                nc.vector._custom_dve(
                    _CTTR,
                    out=scrs[par][:].rearrange("p (c i) -> p c i", i=K),
                    in0=oh3,
                    in1=em_sl_bw[:, None, :].broadcast_to([BL, 2, K]),
                    s0=0.0,
                    s1=-1.0,
                    accum_out=emneg[:],
                )
                nc.vector._custom_dve(
                    _CTTR,
                    out=scr2s[par][:].rearrange("p (c i) -> p c i", i=K),
                    in0=oh3,
                    in1=hist[:, sp1 * K : (sp1 + 1) * K][:, None, :]
                    .broadcast_to([BL, 2, K]),
                    s0=0.0,
                    s1=1.0,
                    accum_out=m8[:, 0:1],
                )
                # transsel[b,i] = trans[i, tag_b] via 2 diagonal 64x64 matmuls
                tsel = psum_pool.tile([BL, K], f32, tag="tsel")
                for r in range(2):
                    nc.tensor.matmul(
                        tsel[64 * r : 64 * r + 64, :],
                        vt[64 * r : 64 * r + 64, :],
                        tmov[64 * r : 64 * r + 64, :],
                        start=True,
                        stop=True,
                        tile_position=(64 * r, 64 * r),
                    )
                # tmp = (hist_s - tsel*(-1) - emneg)*1 = (hist_s+tsel)+em
                # -- bitwise identical to the ref association (sign flips and
                # subtract-of-negation are IEEE-exact).  The max value for
                # max_index is hist_{s+1}[tag] (bitwise == max(tmp)).
                nc.vector.ln_bwd_dx(
                    tmp[:], hist[:, s * K : (s + 1) * K], tsel[:], -1.0,
                    emneg[:], 1.0,
                )
                nc.vector.max_index(
                    t8c[:, tloc * 8 : tloc * 8 + 8], m8[:], tmp[:]
                )

            # compact this chunk's tags (slot stride 8 -> dense) on ScalarE,
            # in two halves so the upper half flushes while the lower half of
            # the chunk is still backtracking (shrinks the end-of-program tail)
            t83 = t8c[:].rearrange("p (s e) -> p s e", e=8)
            h = tc // 2
            nc.scalar.copy(
                tagout[:, c * tc + h : (c + 1) * tc][:, :, None],
                t83[:, h:, 0:1],
            )
            nc.sync.dma_start(
                tags_d[:, c * tc + h : (c + 1) * tc],
                tagout[:, c * tc + h : (c + 1) * tc],
            )
            nc.scalar.copy(
                tagout[:, c * tc : c * tc + h][:, :, None], t83[:, 0:h, 0:1]
            )
            nc.sync.dma_start(
                tags_d[:, c * tc : c * tc + h], tagout[:, c * tc : c * tc + h]
            )
            if c + 1 in tags8_by_chunk:
                del tags8_by_chunk[c + 1]


_NC_CACHE = {}


def _get_nc(t_steps=T, tc=TC, jpool=JPOOL):
    key = (t_steps, tc, jpool)
    if key not in _NC_CACHE:
        _NC_CACHE[key] = build_nc(t_steps, tc, jpool)
    return _NC_CACHE[key]


def make_in_maps(inputs, start_transitions, end_transitions, transitions,
                 t_steps=T):
    """Host-side shard + constant prep. Returns list of per-core input dicts."""
    inputs = np.asarray(inputs, np.float32)
    start = np.asarray(start_transitions, np.float32)
    end = np.asarray(end_transitions, np.float32)
    trans = np.asarray(transitions, np.float32)

    ttb = np.ascontiguousarray(
        np.broadcast_to(trans.T.reshape(1, K * K), (BL, K * K))
    )
    tmov = np.ascontiguousarray(np.tile(trans.T, (4, 1)))
    endt = np.ascontiguousarray(np.broadcast_to(end.reshape(1, K), (BL, K)))
    iota = np.ascontiguousarray(
        np.broadcast_to(np.arange(K, dtype=np.uint32), (BL, K))
    )
    # block-diagonal iota for the 64-wide onehot: row-block r holds 0..31 at
    # free offset 32*(r%2), 255 (never a tag) elsewhere
    iota64 = np.full((BL, 2 * K), 255, dtype=np.uint32)
    for r in range(BL // K):
        off = K * (r % 2)
        iota64[r * K : (r + 1) * K, off : off + K] = np.arange(K, dtype=np.uint32)

    in_maps = []
    for ci in range(NCORES):
        em = np.array(
            inputs[ci * BL : (ci + 1) * BL, :t_steps].reshape(BL, t_steps * K)
        )
        # fold start_transitions into em[0] (same association as the ref)
        em[:, :K] = start.reshape(1, K) + em[:, :K]
        in_maps.append(
            {"em": em, "ttb": ttb, "tmov": tmov, "endt": endt, "iota": iota,
             "iota64": iota64}
        )
    return in_maps


_last_result = None


def kernel(inputs, mask, start_transitions, end_transitions, transitions):
    global _last_result
    mask = np.asarray(mask)
    if not mask.all():
        return _numpy_fallback(
            np.asarray(inputs, np.float32), mask,
            np.asarray(start_transitions, np.float32),
            np.asarray(end_transitions, np.float32),
            np.asarray(transitions, np.float32),
        )

    from concourse.bass_utils import run_bass_kernel_spmd

    nc = _get_nc()
    in_maps = make_in_maps(inputs, start_transitions, end_transitions, transitions)
    res = run_bass_kernel_spmd(nc, in_maps, core_ids=list(range(NCORES)))
    _last_result = res
    tags = np.concatenate([res.results[i]["tags"] for i in range(NCORES)], axis=0)
    return tags.astype(np.int32)


def _numpy_fallback(inputs, mask, start, end, trans):
    """Vectorized numpy Viterbi matching torchcrf/ref semantics (general mask)."""
    em = np.swapaxes(inputs, 0, 1)  # [T, B, K]
    mk = np.swapaxes(mask, 0, 1)  # [T, B]
    nT, nB, nK = em.shape
    score = start[None, :] + em[0]
    hist = np.zeros((nT - 1, nB, nK), np.int32)
    for t in range(1, nT):
        cand = score[:, :, None] + trans[None, :, :] + em[t][:, None, :]
        bp = np.argmax(cand, axis=1).astype(np.int32)
        ns = np.max(cand, axis=1)
        m = mk[t][:, None]
        score = np.where(m, ns, score)
        hist[t - 1] = bp
    score = score + end[None, :]
    tag = np.argmax(score, axis=1).astype(np.int32)
    tags = np.zeros((nT, nB), np.int32)
    tags[nT - 1] = tag
    for t in range(nT - 2, -1, -1):
        prev = np.take_along_axis(hist[t], tag[:, None], axis=1)[:, 0]
        prev = np.where(mk[t + 1], prev, tag)
        tags[t] = prev
        tag = prev
    return np.swapaxes(tags, 0, 1).astype(np.int32)


# revision 26
# speedup vs baseline: 1.0011x; 1.0000x over previous
"""CRF Viterbi decode (torchcrf semantics) on 8 Trainium2 NeuronCores.

Strategy: pure data parallel over batch (1024 rows -> 128 rows/core, one row
per SBUF partition).  Per core:

  Forward (DVE only, scores resident in SBUF, bit-exact vs the jax ref):
    cand[b,j,i] = score[b,i] + trans[i,j]   (stride-0 broadcast TT add)
    premax[b,j] = max_i cand[b,j,i]         (segmented tensor_reduce)
    score'[b,j] = premax[b,j] + em[b,t,j]   (small TT add)
  This 3-op chain is the DVE floor: neuronxcc rejects generic compute on the
  Pool engine, the Activation engine only takes [P,1] bias vectors, fp32 PE
  matmuls run at 4 cycles/row, and no DVE op fuses a tensor-tensor stage
  with a segmented reduce.  Any further op-splitting loses to the ~95ns
  per-op overhead plus the full-vector barrier each step carries.

  Backward (recomputes each step's candidates instead of storing bp):
    A 64-wide block-diagonal one-hot of tag_{s+1} (iota64 holds 0..31 at
    free offset 32*(r%2) for row-block r, 255 elsewhere) is block-transposed
    by the 32x32 vector-transpose, so TWO 64-contraction tile_position
    matmuls gather transsel[b,i] = trans[i, tag_{s+1}(b)] (vs four 32-wide).
    The max value needed by max_index is not recomputed: it equals
    hist_{s+1}[b, tag_{s+1}] bitwise (max-then-add-const == add-const-then-
    max for fp max VALUES), so the per-row scalars are gathered by two DVE
    TTR ops that run inside the PE window:
      emneg[b]  = sum (onehot * em_{s+1-replicated}) * -1
      histsel[b] = sum (onehot * hist_{s+1-replicated}) -> m8 slot 0
    tmp = (hist_s - transsel*(-1) - emneg)*1  (ln_bwd_dx; associations match
    the ref exactly), then max_index against histsel (first-index tie break
    = jnp.argmax).  All max_index inputs are same-engine, so it dispatches
    in-order with no cross-engine event-semaphore wait.

Inputs are taken at full shape; sharding/gather happens on host inside
kernel().
"""

import sys

import numpy as np

if "/opt/trn_rl_repo" not in sys.path:
    sys.path.insert(0, "/opt/trn_rl_repo")

B, T, K = 1024, 1024, 32
NCORES = 8
BL = B // NCORES  # 128 batch rows per core
TC = 64  # time chunk (em streaming / tags8 chunking)
POS_BIG = 3.0e38

# forward j-split: DVE owns j in [0, JD), Pool owns j in [JD, K).
# NOTE: neuronxcc rejects generic compute (TensorTensor/TensorScalarPtr) on
# the Pool engine, so jpool must stay 0 on real hardware; the split code is
# kept for cost-model experiments only.
JPOOL = 0


def build_nc(t_steps: int = T, tc: int = TC, jpool: int = JPOOL):
    """Build + compile the per-core Bass program (same NEFF on all 8 cores)."""
    import concourse.bass as bass
    import concourse.tile as tile
    from concourse import bacc, mybir

    f32 = mybir.dt.float32
    u32 = mybir.dt.uint32
    i32 = mybir.dt.int32
    Alu = mybir.AluOpType
    Ax = mybir.AxisListType

    nsteps = t_steps
    nchunks = (nsteps + tc - 1) // tc
    assert nsteps % tc == 0

    nc = bacc.Bacc(
        "TRN2", target_bir_lowering=False, debug=False, enable_asserts=False
    )

    em_d = nc.dram_tensor("em", [BL, nsteps * K], f32, kind="ExternalInput").ap()
    ttb_d = nc.dram_tensor("ttb", [BL, K * K], f32, kind="ExternalInput").ap()
    tmov_d = nc.dram_tensor("tmov", [128, K], f32, kind="ExternalInput").ap()
    endt_d = nc.dram_tensor("endt", [BL, K], f32, kind="ExternalInput").ap()
    iota_d = nc.dram_tensor("iota", [BL, K], u32, kind="ExternalInput").ap()
    # block-diagonal iota: row-block r holds 0..31 at free offset 32*(r%2),
    # 255 elsewhere -> onehot64 is block-diagonal, so one 64-contraction
    # matmul covers two row blocks (2 matmuls/step instead of 4)
    iota64_d = nc.dram_tensor("iota64", [BL, 2 * K], u32, kind="ExternalInput").ap()
    tags_d = nc.dram_tensor("tags", [BL, nsteps], i32, kind="ExternalOutput").ap()

    with tile.TileContext(nc) as tc_ctx:
        _body(nc, tc_ctx, bass, mybir, Alu, Ax, f32, u32, i32,
              em_d, ttb_d, tmov_d, endt_d, iota_d, iota64_d, tags_d, nsteps,
              tc, nchunks, jpool)

    nc.compile()
    return nc


def _body(nc, tc_ctx, bass, mybir, Alu, Ax, f32, u32, i32,
          em_d, ttb_d, tmov_d, endt_d, iota_d, iota64_d, tags_d, nsteps,
          tc, nchunks, jpool):
    from contextlib import ExitStack

    from concourse.dve_ops import TENSOR_TENSOR_REDUCE as _CTTR

    jd = K - jpool  # DVE-owned j count

    ctx = ExitStack()
    with ctx:
        const_pool = ctx.enter_context(tc_ctx.tile_pool(name="const", bufs=1))
        hist_pool = ctx.enter_context(tc_ctx.tile_pool(name="hist", bufs=1))
        em_pool = ctx.enter_context(tc_ctx.tile_pool(name="em", bufs=2))
        work_pool = ctx.enter_context(tc_ctx.tile_pool(name="work", bufs=1))
        tags8_pool = ctx.enter_context(tc_ctx.tile_pool(name="tags8", bufs=3))
        psum_pool = ctx.enter_context(
            tc_ctx.tile_pool(name="psum", bufs=2, space="PSUM")
        )

        # ---- constants ----
        # em chunk 0 is what step t=0 needs first, then ttb for t=1; the rest
        # (tmov/endt/iota*) is only read ~3ms later by the backward, so issue
        # the forward-critical transfers first.
        emf0 = em_pool.tile([BL, tc * K], f32, tag="emchunk")
        nc.sync.dma_start(emf0[:], em_d[:, 0 : tc * K])
        ttb = const_pool.tile([BL, K * K], f32)  # ttb[b, j*K+i] = trans[i, j]
        nc.sync.dma_start(ttb[:], ttb_d[:])
        tmov = const_pool.tile([128, K], f32)  # trans.T tiled x4 over partitions
        nc.sync.dma_start(tmov[:], tmov_d[:])
        endt = const_pool.tile([BL, K], f32)
        nc.sync.dma_start(endt[:], endt_d[:])
        iota = const_pool.tile([BL, K], u32)
        nc.sync.dma_start(iota[:], iota_d[:])
        iota64 = const_pool.tile([BL, 2 * K], u32)
        nc.sync.dma_start(iota64[:], iota64_d[:])

        # ---- working tiles ----
        hist = hist_pool.tile([BL, nsteps * K], f32)  # all forward scores
        cand = work_pool.tile([BL, max(jd, 1) * K], f32)  # DVE j-slice
        candp = work_pool.tile([BL, max(jpool, 1) * K], f32)  # Pool j-slice
        l1 = work_pool.tile([BL, max(jpool, 1) * (K // 2)], f32)
        premax = work_pool.tile([BL, K], f32)
        # double-buffered by step parity: breaks the Pool-write-after-DVE-read
        # serialization on the per-step scalar tiles
        m8s = [work_pool.tile([BL, 8], f32, name=f"m8_{i}") for i in range(2)]
        tmps = [work_pool.tile([BL, K], f32, name=f"tmp_{i}") for i in range(2)]
        emnegs = [work_pool.tile([BL, 1], f32, name=f"emneg_{i}") for i in range(2)]
        scrs = [work_pool.tile([BL, 2 * K], f32, name=f"scr_{i}") for i in range(2)]
        scr2s = [work_pool.tile([BL, 2 * K], f32, name=f"scr2_{i}") for i in range(2)]
        onehots = [work_pool.tile([BL, 2 * K], f32, name=f"oh_{i}") for i in range(2)]
        vts = [work_pool.tile([BL, 2 * K], f32, name=f"vt_{i}") for i in range(2)]
        tagout = work_pool.tile([BL, nsteps], i32)

        nc.vector.memset(m8s[0][:], POS_BIG)
        nc.vector.memset(m8s[1][:], POS_BIG)

        ttb3 = ttb[:].rearrange("p (j i) -> p j i", i=K)
        cand3 = cand[:].rearrange("p (j i) -> p j i", i=K)
        candp3 = candp[:].rearrange("p (j i) -> p j i", i=K)
        l13 = l1[:].rearrange("p (j i) -> p j i", i=K // 2)

        # ================= forward =================
        for c in range(nchunks):
            if c == 0:
                emf = emf0
            else:
                emf = em_pool.tile([BL, tc * K], f32, tag="emchunk")
                nc.sync.dma_start(emf[:], em_d[:, c * tc * K : (c + 1) * tc * K])
            for tloc in range(tc):
                t = c * tc + tloc
                em_sl = emf[:, tloc * K : (tloc + 1) * K]
                h_t = hist[:, t * K : (t + 1) * K]
                if t == 0:
                    nc.vector.tensor_copy(h_t, em_sl)
                    continue
                h_prev = hist[:, (t - 1) * K : t * K]
                h_bc = h_prev[:, None, :]
                if jpool > 0:
                    # Pool: cand slice [jd, K) + pairwise L1 max to 16-wide
                    nc.gpsimd.scalar_tensor_tensor(
                        candp3[:, 0:jpool, :],
                        h_bc.broadcast_to([BL, jpool, K]),
                        0.0,
                        ttb3[:, jd:K, :],
                        Alu.bypass,
                        Alu.add,
                    )
                    nc.gpsimd.tensor_tensor(
                        l13[:, 0:jpool, :],
                        candp3[:, 0:jpool, 0 : K // 2],
                        candp3[:, 0:jpool, K // 2 : K],
                        Alu.max,
                    )
                if jd > 0:
                    nc.vector.tensor_tensor(
                        cand3[:, 0:jd, :],
                        h_bc.broadcast_to([BL, jd, K]),
                        ttb3[:, 0:jd, :],
                        Alu.add,
                    )
                    nc.vector.tensor_reduce(
                        premax[:, 0:jd], cand3[:, 0:jd, :], Ax.X, Alu.max
                    )
                if jpool > 0:
                    nc.vector.tensor_reduce(
                        premax[:, jd:K], l13[:, 0:jpool, :], Ax.X, Alu.max
                    )
                nc.vector.tensor_tensor(h_t, premax[:], em_sl, Alu.add)

        # ================= final argmax =================
        # ref: score = hist[T-1] + end_transitions, then argmax (first index)
        tags8_cur = tags8_pool.tile([BL, tc * 8], u32, tag="t8")
        tmp0 = tmps[(nsteps - 1) % 2]
        m80 = m8s[(nsteps - 1) % 2]
        nc.vector.tensor_tensor(
            tmp0[:], hist[:, (nsteps - 1) * K : nsteps * K], endt[:], Alu.add
        )
        nc.vector.tensor_reduce(m80[:, 0:1], tmp0[:], Ax.X, Alu.max)
        last_slot = (nsteps - 1) - (nchunks - 1) * tc
        nc.vector.max_index(
            tags8_cur[:, last_slot * 8 : last_slot * 8 + 8], m80[:], tmp0[:]
        )

        # ================= backward =================
        tags8_by_chunk = {nchunks - 1: tags8_cur}
        for c in range(nchunks - 1, -1, -1):
            # em[s+1] for s in [c*tc, (c+1)*tc): dram slice offset by one step
            n_em = tc if c < nchunks - 1 else tc - 1
            embw = em_pool.tile([BL, tc * K], f32, tag="emchunk")
            nc.sync.dma_start(
                embw[:, : n_em * K],
                em_d[:, (c * tc + 1) * K : (c * tc + 1 + n_em) * K],
            )
            if c not in tags8_by_chunk:
                tags8_by_chunk[c] = tags8_pool.tile(
                    [BL, tc * 8], u32, tag="t8", name=f"t8c{c}"
                )
            t8c = tags8_by_chunk[c]

            s_hi = min(nsteps - 2, (c + 1) * tc - 1)
            for s in range(s_hi, c * tc - 1, -1):
                tloc = s - c * tc
                par = s % 2
                onehot, vt = onehots[par], vts[par]
                tmp, m8, emneg = tmps[par], m8s[par], emnegs[par]
                # one-hot of tag_{s+1}
                sp1 = s + 1
                cp1 = sp1 // tc
                t8p = tags8_by_chunk[cp1]
                slot = sp1 - cp1 * tc
                # block-diagonal one-hot of tag_{s+1} ([BL, 64]; row-block r
                # has its 32 valid lanes at free offset 32*(r%2), zeros
                # elsewhere since iota64 is 255 there)
                nc.vector.tensor_tensor(
                    onehot[:],
                    iota64[:],
                    t8p[:, slot * 8 : slot * 8 + 1].broadcast_to([BL, 2 * K]),
                    Alu.is_equal,
                )
                nc.vector.transpose(vt[:], onehot[:])
                # per-row scalars (DVE TTRs, run inside the PE window), using
                # the j-replicated view of em/hist so the 255-half contributes
                # exact zeros:
                # emneg[b] = -em_{s+1}[b, tag];
                # histsel[b] = hist_{s+1}[b, tag] -> m8 slot 0 (bitwise equal
                # to max(tmp), so the separate tensor_reduce is not needed, and
                # max_index has no cross-engine dep so it dispatches in-order)
                oh3 = onehot[:].rearrange("p (c i) -> p c i", i=K)
                em_sl_bw = embw[:, tloc * K : (tloc + 1) * K]
                nc.vector._custom_dve(
                    _CTTR,
                    out=scrs[par][:].rearrange("p (c i) -> p c i", i=K),
                    in0=oh3,
                    in1=em_sl_bw[:, None, :].broadcast_to([BL, 2, K]),
                    s0=0.0,
                    s1=-1.0,
                    accum_out=emneg[:],
                )
                nc.vector._custom_dve(
                    _CTTR,
                    out=scr2s[par][:].rearrange("p (c i) -> p c i", i=K),
                    in0=oh3,
                    in1=hist[:, sp1 * K : (sp1 + 1) * K][:, None, :]
                    .broadcast_to([BL, 2, K]),
                    s0=0.0,
                    s1=1.0,
                    accum_out=m8[:, 0:1],
                )
                # transsel[b,i] = trans[i, tag_b] via 2 diagonal 64x64 matmuls
                tsel = psum_pool.tile([BL, K], f32, tag="tsel")
                for r in range(2):
                    nc.tensor.matmul(
                        tsel[64 * r : 64 * r + 64, :],
                        vt[64 * r : 64 * r + 64, :],
                        tmov[64 * r : 64 * r + 64, :],
                        start=True,
                        stop=True,
                        tile_position=(64 * r, 64 * r),
                    )
                # tmp = (hist_s - tsel*(-1) - emneg)*1 = (hist_s+tsel)+em
                # -- bitwise identical to the ref association (sign flips and
                # subtract-of-negation are IEEE-exact).  The max value for
                # max_index is hist_{s+1}[tag] (bitwise == max(tmp)).
                nc.vector.ln_bwd_dx(
                    tmp[:], hist[:, s * K : (s + 1) * K], tsel[:], -1.0,
                    emneg[:], 1.0,
                )
                nc.vector.max_index(
                    t8c[:, tloc * 8 : tloc * 8 + 8], m8[:], tmp[:]
                )

            # compact this chunk's tags (slot stride 8 -> dense) on ScalarE,
            # in two halves so the upper half flushes while the lower half of
            # the chunk is still backtracking (shrinks the end-of-program tail)
            t83 = t8c[:].rearrange("p (s e) -> p s e", e=8)
            h = tc // 2
            nc.scalar.copy(
                tagout[:, c * tc + h : (c + 1) * tc][:, :, None],
                t83[:, h:, 0:1],
            )
            nc.sync.dma_start(
                tags_d[:, c * tc + h : (c + 1) * tc],
                tagout[:, c * tc + h : (c + 1) * tc],
            )
            nc.scalar.copy(
                tagout[:, c * tc : c * tc + h][:, :, None], t83[:, 0:h, 0:1]
            )
            nc.sync.dma_start(
                tags_d[:, c * tc : c * tc + h], tagout[:, c * tc : c * tc + h]
            )
            if c + 1 in tags8_by_chunk:
                del tags8_by_chunk[c + 1]


_NC_CACHE = {}


def _get_nc(t_steps=T, tc=TC, jpool=JPOOL):
    key = (t_steps, tc, jpool)
    if key not in _NC_CACHE:
        _NC_CACHE[key] = build_nc(t_steps, tc, jpool)
    return _NC_CACHE[key]


def make_in_maps(inputs, start_transitions, end_transitions, transitions,
                 t_steps=T):
    """Host-side shard + constant prep. Returns list of per-core input dicts."""
    inputs = np.asarray(inputs, np.float32)
    start = np.asarray(start_transitions, np.float32)
    end = np.asarray(end_transitions, np.float32)
    trans = np.asarray(transitions, np.float32)

    ttb = np.ascontiguousarray(
        np.broadcast_to(trans.T.reshape(1, K * K), (BL, K * K))
    )
    tmov = np.ascontiguousarray(np.tile(trans.T, (4, 1)))
    endt = np.ascontiguousarray(np.broadcast_to(end.reshape(1, K), (BL, K)))
    iota = np.ascontiguousarray(
        np.broadcast_to(np.arange(K, dtype=np.uint32), (BL, K))
    )
    # block-diagonal iota for the 64-wide onehot: row-block r holds 0..31 at
    # free offset 32*(r%2), 255 (never a tag) elsewhere
    iota64 = np.full((BL, 2 * K), 255, dtype=np.uint32)
    for r in range(BL // K):
        off = K * (r % 2)
        iota64[r * K : (r + 1) * K, off : off + K] = np.arange(K, dtype=np.uint32)

    in_maps = []
    for ci in range(NCORES):
        em = np.array(
            inputs[ci * BL : (ci + 1) * BL, :t_steps].reshape(BL, t_steps * K)
        )
        # fold start_transitions into em[0] (same association as the ref)
        em[:, :K] = start.reshape(1, K) + em[:, :K]
        in_maps.append(
            {"em": em, "ttb": ttb, "tmov": tmov, "endt": endt, "iota": iota,
             "iota64": iota64}
        )
    return in_maps


_last_result = None


def kernel(inputs, mask, start_transitions, end_transitions, transitions):
    global _last_result
    mask = np.asarray(mask)
    if not mask.all():
        return _numpy_fallback(
            np.asarray(inputs, np.float32), mask,
            np.asarray(start_transitions, np.float32),
            np.asarray(end_transitions, np.float32),
            np.asarray(transitions, np.float32),
        )

    from concourse.bass_utils import run_bass_kernel_spmd

    nc = _get_nc()
    in_maps = make_in_maps(inputs, start_transitions, end_transitions, transitions)
    res = run_bass_kernel_spmd(nc, in_maps, core_ids=list(range(NCORES)))
    _last_result = res
    tags = np.concatenate([res.results[i]["tags"] for i in range(NCORES)], axis=0)
    return tags.astype(np.int32)


def _numpy_fallback(inputs, mask, start, end, trans):
    """Vectorized numpy Viterbi matching torchcrf/ref semantics (general mask)."""
    em = np.swapaxes(inputs, 0, 1)  # [T, B, K]
    mk = np.swapaxes(mask, 0, 1)  # [T, B]
    nT, nB, nK = em.shape
    score = start[None, :] + em[0]
    hist = np.zeros((nT - 1, nB, nK), np.int32)
    for t in range(1, nT):
        cand = score[:, :, None] + trans[None, :, :] + em[t][:, None, :]
        bp = np.argmax(cand, axis=1).astype(np.int32)
        ns = np.max(cand, axis=1)
        m = mk[t][:, None]
        score = np.where(m, ns, score)
        hist[t - 1] = bp
    score = score + end[None, :]
    tag = np.argmax(score, axis=1).astype(np.int32)
    tags = np.zeros((nT, nB), np.int32)
    tags[nT - 1] = tag
    for t in range(nT - 2, -1, -1):
        prev = np.take_along_axis(hist[t], tag[:, None], axis=1)[:, 0]
        prev = np.where(mk[t + 1], prev, tag)
        tags[t] = prev
        tag = prev
    return np.swapaxes(tags, 0, 1).astype(np.int32)
